# revision 11
# baseline (speedup 1.0000x reference)
"""DAP MSA layer (PWA + SwiGLU transition + outer-product-mean) on 8 trn2 cores.

Three SPMD launches:
  L1 (row-sharded over N):  w_att = softmax(proj_z(norm(z)) + mask)
  L2 (seq-sharded over S):  full m pipeline -> m_new, a, b
  L3 (row-sharded over N):  outer-product-mean + out-proj -> z_new
Matmuls in bf16 (fp32 PSUM accumulate), element-wise/LN/softmax in fp32.
"""
import numpy as np
import ml_dtypes
from contextlib import ExitStack

import concourse.bass as bass
import concourse.tile as tile
from concourse import bacc, mybir
from concourse.bass_utils import run_bass_kernel_spmd
from concourse.masks import make_identity

F32 = mybir.dt.float32
BF16 = mybir.dt.bfloat16
AX = mybir.AxisListType.X
ALU = mybir.AluOpType
ACTF = mybir.ActivationFunctionType
BF = ml_dtypes.bfloat16

B, S, N = 1, 1024, 384
CZ, CM, H, CH, COPM, HID = 128, 64, 8, 32, 32, 256
NCORE = 8
NI = N // NCORE      # 48 z-rows per core (L1, L3)
SL = S // NCORE      # 128 msa-rows per core (L2)
SC = 16              # s-rows per L2 chunk
T = SC * 3           # 48 token-tiles per L2 chunk
HD = H * CH          # 256


def _ln_stats(nc, pool, xbuf, t_cnt, c_dim, eps):
    """xbuf [128, t_cnt, c_dim] f32 -> (negmu, rstd) each [128, t_cnt]."""
    negsum = pool.tile([128, t_cnt], F32, tag="st_a")
    nc.vector.tensor_reduce(negsum, xbuf[:, :, :], axis=AX, op=ALU.add, negate=True)
    sq = pool.tile([128, t_cnt * c_dim], BF16, tag="st_sq")
    nc.scalar.activation(sq, xbuf.rearrange("p t c -> p (t c)"), ACTF.Square)
    s2 = pool.tile([128, t_cnt], F32, tag="st_b")
    nc.vector.tensor_reduce(s2, sq.rearrange("p (t c) -> p t c", c=c_dim),
                            axis=AX, op=ALU.add)
    negmu = pool.tile([128, t_cnt], F32, tag="st_c")
    nc.vector.tensor_scalar_mul(negmu, negsum, 1.0 / c_dim)
    var = pool.tile([128, t_cnt], F32, tag="st_d")
    nc.vector.tensor_scalar_mul(var, s2, 1.0 / c_dim)
    mu2 = pool.tile([128, t_cnt], F32, tag="st_e")
    nc.vector.tensor_mul(mu2, negmu, negmu)
    nc.vector.tensor_tensor(var, var, mu2, op=ALU.subtract)
    sd = pool.tile([128, t_cnt], F32, tag="st_f")
    nc.scalar.activation(sd, var, ACTF.Sqrt, bias=eps)
    rstd = pool.tile([128, t_cnt], F32, tag="st_g")
    nc.vector.reciprocal(rstd, sd)
    return negmu, rstd


def build_l1():
    nc = bacc.Bacc("TRN2", target_bir_lowering=False, debug=False, num_devices=NCORE)
    TOK = NI * N
    zr = nc.dram_tensor("zr", [TOK, CZ], F32, kind="ExternalInput").ap()
    mb = nc.dram_tensor("mb", [TOK], F32, kind="ExternalInput").ap()
    wz = nc.dram_tensor("wz", [CZ, H], BF16, kind="ExternalInput").ap()
    bz = nc.dram_tensor("bz", [H, 1], F32, kind="ExternalInput").ap()
    watt = nc.dram_tensor("watt", [H, TOK], F32, kind="ExternalOutput").ap()

    zt = zr.rearrange("(t p) c -> p t c", p=128)
    NT = 12
    NCH = (TOK // 128) // NT  # 12

    with tile.TileContext(nc) as tc, ExitStack() as ctx:
        const = ctx.enter_context(tc.tile_pool(name="const", bufs=1))
        pool = ctx.enter_context(tc.tile_pool(name="p", bufs=2))
        big = ctx.enter_context(tc.tile_pool(name="big", bufs=1))
        ps = ctx.enter_context(tc.tile_pool(name="ps", bufs=4, space="PSUM"))
        pst = ctx.enter_context(tc.tile_pool(name="pst", bufs=2, space="PSUM"))

        ident = const.tile([128, 128], BF16)
        make_identity(nc, ident)
        wz_sb = const.tile([CZ, H], BF16)
        nc.sync.dma_start(wz_sb, wz)
        bz_sb = const.tile([H, 1], F32)
        nc.sync.dma_start(bz_sb, bz)
        eps = const.tile([128, 1], F32)
        nc.vector.memset(eps, 1e-5)

        logits = big.tile([H, TOK], F32)

        for ch in range(NCH):
            zc = pool.tile([128, NT, CZ], F32, tag="zc")
            nc.sync.dma_start(zc, zt[:, ch * NT:(ch + 1) * NT, :])
            negmu, rstd = _ln_stats(nc, pool, zc, NT, CZ, eps)
            xh = pool.tile([128, NT, CZ], BF16, tag="xh")
            for t in range(NT):
                nc.vector.tensor_scalar(xh[:, t, :], zc[:, t, :],
                                        negmu[:, t:t + 1], rstd[:, t:t + 1],
                                        op0=ALU.add, op1=ALU.mult)
            xhT = pool.tile([128, NT, 128], BF16, tag="xhT")
            for t in range(NT):
                pt = pst.tile([128, 128], BF16, tag="pt")
                nc.tensor.transpose(pt, xh[:, t, :], ident)
                nc.any.tensor_copy(xhT[:, t, :], pt)
            # mask-bias chunk, broadcast to 8 partitions
            mbc = pool.tile([H, NT * 128], F32, tag="mbc")
            mb_sl = mb[ch * NT * 128:(ch + 1) * NT * 128]
            mb_b = bass.AP(tensor=mb_sl.tensor, offset=mb_sl.offset,
                           ap=[[0, H]] + list(mb_sl.ap))
            nc.gpsimd.dma_start(out=mbc, in_=mb_b)
            xhT_f = xhT.rearrange("p t f -> p (t f)")
            for g in range(NT // 4):
                pb = ps.tile([H, 512], F32, tag="pb")
                nc.tensor.matmul(pb, wz_sb, xhT_f[:, g * 512:(g + 1) * 512],
                                 start=True, stop=True)
                off = ch * NT * 128 + g * 512
                nc.vector.tensor_scalar_add(logits[:, off:off + 512], pb, bz_sb)
                nc.vector.tensor_tensor(logits[:, off:off + 512],
                                        logits[:, off:off + 512],
                                        mbc[:, g * 512:(g + 1) * 512], op=ALU.add)

        ex = big.tile([H, TOK], F32)
        nc.scalar.activation(ex, logits, ACTF.Exp)
        ex_v = ex.rearrange("h (i j) -> h i j", j=N)
        sums = pool.tile([H, NI], F32, tag="sm")
        nc.vector.tensor_reduce(sums, ex_v, axis=AX, op=ALU.add)
        rec = pool.tile([H, NI], F32, tag="rc")
        nc.vector.reciprocal(rec, sums)
        for i in range(NI):
            nc.vector.tensor_scalar_mul(ex_v[:, i, :], ex_v[:, i, :],
                                        rec[:, i:i + 1])
        nc.sync.dma_start(watt, ex)
    nc.compile()
    return nc


def build_l2():
    nc = bacc.Bacc("TRN2", target_bir_lowering=False, debug=False, num_devices=NCORE)
    TOKL = SL * N
    ms = nc.dram_tensor("ms", [TOKL, CM], F32, kind="ExternalInput").ap()
    wt = nc.dram_tensor("wt", [128, 3 * H * N], BF16, kind="ExternalInput").ap()
    wvg = nc.dram_tensor("wvg", [CM, 2 * HD], BF16, kind="ExternalInput").ap()
    wo = nc.dram_tensor("wo", [HD, CM], BF16, kind="ExternalInput").ap()
    w12 = nc.dram_tensor("w12", [CM, 2 * HID], BF16, kind="ExternalInput").ap()
    w3 = nc.dram_tensor("w3", [HID, CM], BF16, kind="ExternalInput").ap()
    wab = nc.dram_tensor("wab", [CM, 2 * COPM], BF16, kind="ExternalInput").ap()
    mo_d = nc.dram_tensor("mo_d", [TOKL, CM], F32, kind="ExternalOutput").ap()
    a_d = nc.dram_tensor("a_d", [SL, COPM, N], BF16, kind="ExternalOutput").ap()
    b_d = nc.dram_tensor("b_d", [SL, COPM, N], BF16, kind="ExternalOutput").ap()

    mt = ms.rearrange("(t p) c -> p t c", p=128)
    mot = mo_d.rearrange("(t p) c -> p t c", p=128)

    with tile.TileContext(nc) as tc, ExitStack() as ctx:
        const = ctx.enter_context(tc.tile_pool(name="const", bufs=1))
        pool = ctx.enter_context(tc.tile_pool(name="p", bufs=2))
        buf1 = ctx.enter_context(tc.tile_pool(name="b1", bufs=1))
        ps = ctx.enter_context(tc.tile_pool(name="ps", bufs=2, space="PSUM"))
        pst = ctx.enter_context(tc.tile_pool(name="pst", bufs=2, space="PSUM"))

        ident = const.tile([128, 128], BF16)
        make_identity(nc, ident)
        eps = const.tile([128, 1], F32)
        nc.vector.memset(eps, 1e-5)
        wvg_sb = const.tile([CM, 2 * HD], BF16)
        nc.sync.dma_start(wvg_sb, wvg)
        wo_sb = const.tile([128, 2, CM], BF16)
        nc.sync.dma_start(wo_sb, wo.rearrange("(k p) c -> p k c", p=128))
        w12_sb = const.tile([CM, 2 * HID], BF16)
        nc.sync.dma_start(w12_sb, w12)
        w3_sb = const.tile([128, 2, CM], BF16)
        nc.sync.dma_start(w3_sb, w3.rearrange("(k p) c -> p k c", p=128))
        wab_sb = const.tile([CM, 2 * COPM], BF16)
        nc.sync.dma_start(wab_sb, wab)
        wt_sb = const.tile([128, 3, H, N], BF16)
        nc.sync.dma_start(wt_sb, wt.rearrange("p (c h i) -> p c h i", c=3, h=H))

        def ln_block(xbuf, out_bf, out_T):
            negmu, rstd = _ln_stats(nc, pool, xbuf, T, CM, eps)
            for t in range(T):
                nc.vector.tensor_scalar(out_bf[:, t, :], xbuf[:, t, :],
                                        negmu[:, t:t + 1], rstd[:, t:t + 1],
                                        op0=ALU.add, op1=ALU.mult)
            for t in range(T):
                pt = pst.tile([CM, 128], BF16, tag="pt")
                nc.tensor.transpose(pt, out_bf[:, t, :], ident)
                nc.any.tensor_copy(out_T[:, t, :], pt)

        for ch in range(SL // SC):
            mbuf = pool.tile([128, T, CM], F32, tag="mbuf")
            nc.sync.dma_start(mbuf, mt[:, ch * T:(ch + 1) * T, :])
            xh = buf1.tile([128, T, CM], BF16, tag="xh")
            mnT = buf1.tile([CM, T, 128], BF16, tag="mnT")
            ln_block(mbuf, xh, mnT)

            # ---- v | gate ----
            vbuf = buf1.tile([128, 3, SC, HD], BF16, tag="vbuf")
            gbuf = buf1.tile([128, 3, SC, HD], BF16, tag="gbuf")
            for sl in range(SC):
                for ic in range(3):
                    pv = ps.tile([128, 512], F32, tag="mm")
                    nc.tensor.matmul(pv, mnT[:, sl * 3 + ic, :], wvg_sb,
                                     start=True, stop=True)
                    nc.vector.tensor_copy(vbuf[:, ic, sl, :], pv[:, :HD])
                    nc.scalar.activation(gbuf[:, ic, sl, :], pv[:, HD:],
                                         ACTF.Sigmoid)

            # ---- o = att @ v ; go = gate*o ; transpose go ----
            goT = buf1.tile([128, 2, SC, 3, 128], BF16, tag="goT")
            for ic in range(3):
                go = buf1.tile([128, SC, HD], BF16, tag="go")
                for hh in range(H):
                    po = ps.tile([128, 512], F32, tag="mm")
                    for jc in range(3):
                        nc.tensor.matmul(
                            po, wt_sb[:, jc, hh, ic * 128:(ic + 1) * 128],
                            vbuf[:, jc, :, hh * CH:(hh + 1) * CH],
                            start=(jc == 0), stop=(jc == 2))
                    nc.vector.tensor_tensor(
                        go[:, :, hh * CH:(hh + 1) * CH],
                        po.rearrange("p (s d) -> p s d", d=CH),
                        gbuf[:, ic, :, hh * CH:(hh + 1) * CH], op=ALU.mult)
                for sl in range(SC):
                    ptT = pst.tile([128, 2, 128], BF16, tag="pt")
                    nc.tensor.transpose(ptT[:, 0, :], go[:, sl, 0:128], ident)
                    nc.tensor.transpose(ptT[:, 1, :], go[:, sl, 128:256], ident)
                    nc.any.tensor_copy(goT[:, 0, sl, ic, :], ptT[:, 0, :])
                    nc.any.tensor_copy(goT[:, 1, sl, ic, :], ptT[:, 1, :])

            # ---- m1 = m + go @ woT ----
            m1 = buf1.tile([128, T, CM], F32, tag="m1")
            goT_f = goT.rearrange("p h s c f -> p h (s c f)")
            for n in range(12):
                pd = ps.tile([CM, 512], F32, tag="mmd")
                for k in range(2):
                    nc.tensor.matmul(pd, wo_sb[:, k, :],
                                     goT_f[:, k, n * 512:(n + 1) * 512],
                                     start=(k == 0), stop=(k == 1))
                dsb = pool.tile([CM, 512], BF16, tag="dsb")
                nc.any.tensor_copy(dsb, pd)
                ptb = pst.tile([128, 4, CM], BF16, tag="pt")
                for q in range(4):
                    nc.tensor.transpose(ptb[:, q, :], dsb[:, q * 128:(q + 1) * 128],
                                        ident[:CM, :CM])
                nc.vector.tensor_tensor(m1[:, n * 4:(n + 1) * 4, :], ptb,
                                        mbuf[:, n * 4:(n + 1) * 4, :], op=ALU.add)

            # ---- transition ----
            xh2 = buf1.tile([128, T, CM], BF16, tag="xh")
            tT = buf1.tile([CM, T, 128], BF16, tag="mnT")
            ln_block(m1, xh2, tT)
            tT_f = tT.rearrange("c t f -> c (t f)")
            hm = buf1.tile([128, 2, T * 128], BF16, tag="vbuf")
            for g in range(12):
                sw = pool.tile([128, 2, 512], BF16, tag="sw")
                for hc in range(2):
                    pf = ps.tile([128, 512], F32, tag="mm")
                    nc.tensor.matmul(pf, w12_sb[:, hc * 128:(hc + 1) * 128],
                                     tT_f[:, g * 512:(g + 1) * 512],
                                     start=True, stop=True)
                    nc.scalar.activation(sw[:, hc, :], pf, ACTF.Silu)
                for hc in range(2):
                    pf = ps.tile([128, 512], F32, tag="mm")
                    nc.tensor.matmul(pf, w12_sb[:, 256 + hc * 128:256 + (hc + 1) * 128],
                                     tT_f[:, g * 512:(g + 1) * 512],
                                     start=True, stop=True)
                    nc.vector.tensor_tensor(hm[:, hc, g * 512:(g + 1) * 512],
                                            sw[:, hc, :], pf, op=ALU.mult)
            m2 = pool.tile([128, T, CM], F32, tag="mbuf")
            for n in range(12):
                pd = ps.tile([CM, 512], F32, tag="mmd")
                for k in range(2):
                    nc.tensor.matmul(pd, w3_sb[:, k, :],
                                     hm[:, k, n * 512:(n + 1) * 512],
                                     start=(k == 0), stop=(k == 1))
                dsb = pool.tile([CM, 512], BF16, tag="dsb")
                nc.any.tensor_copy(dsb, pd)
                ptb = pst.tile([128, 4, CM], BF16, tag="pt")
                for q in range(4):
                    nc.tensor.transpose(ptb[:, q, :], dsb[:, q * 128:(q + 1) * 128],
                                        ident[:CM, :CM])
                nc.vector.tensor_tensor(m2[:, n * 4:(n + 1) * 4, :], ptb,
                                        m1[:, n * 4:(n + 1) * 4, :], op=ALU.add)
            nc.sync.dma_start(mot[:, ch * T:(ch + 1) * T, :], m2)

            # ---- a|b ----
            xh3 = buf1.tile([128, T, CM], BF16, tag="xh")
            moT = buf1.tile([CM, T, 128], BF16, tag="mnT")
            ln_block(m2, xh3, moT)
            moT_f = moT.rearrange("c t f -> c (t f)")
            ab = buf1.tile([2 * COPM, T * 128], BF16, tag="gbuf")
            for g in range(12):
                pa = ps.tile([2 * COPM, 512], F32, tag="mmd")
                nc.tensor.matmul(pa, wab_sb, moT_f[:, g * 512:(g + 1) * 512],
                                 start=True, stop=True)
                nc.any.tensor_copy(ab[:, g * 512:(g + 1) * 512], pa)
            ab_v = ab.rearrange("c (s i) -> c s i", i=N)
            nc.sync.dma_start(
                a_d[ch * SC:(ch + 1) * SC, :, :].rearrange("s c i -> c s i"),
                ab_v[0:COPM, :, :])
            nc.sync.dma_start(
                b_d[ch * SC:(ch + 1) * SC, :, :].rearrange("s c i -> c s i"),
                ab_v[COPM:, :, :])
    nc.compile()
    return nc


def build_l3():
    nc = bacc.Bacc("TRN2", target_bir_lowering=False, debug=False, num_devices=NCORE)
    TOK = NI * N
    a_s = nc.dram_tensor("a_s", [S, COPM * NI], BF16, kind="ExternalInput").ap()
    b_h = nc.dram_tensor("b_h", [S, COPM, N], BF16, kind="ExternalInput").ap()
    zr = nc.dram_tensor("zr", [TOK, CZ], F32, kind="ExternalInput").ap()
    wz = nc.dram_tensor("wz", [COPM * COPM, CZ], BF16, kind="ExternalInput").ap()
    bo = nc.dram_tensor("bo", [CZ, 1], F32, kind="ExternalInput").ap()
    outer_d = nc.dram_tensor("outer_d", [COPM * NI, COPM * N], BF16)
    zo = nc.dram_tensor("zo", [TOK, CZ], F32, kind="ExternalOutput").ap()

    zt = zr.rearrange("(t p) c -> p t c", p=128)
    zot = zo.rearrange("(t p) c -> p t c", p=128)

    with tile.TileContext(nc) as tc, ExitStack() as ctx:
        const = ctx.enter_context(tc.tile_pool(name="const", bufs=1))
        ps = ctx.enter_context(tc.tile_pool(name="ps", bufs=3, space="PSUM"))
        pst = ctx.enter_context(tc.tile_pool(name="pst", bufs=2, space="PSUM"))

        ident = const.tile([128, 128], BF16)
        make_identity(nc, ident)
        wz_sb = const.tile([128, 8, CZ], BF16)
        nc.sync.dma_start(wz_sb, wz.rearrange("(k p) z -> p k z", p=128))
        bo_sb = const.tile([CZ, 1], F32)
        nc.sync.dma_start(bo_sb, bo)
        a_sb = const.tile([128, 8, COPM * NI], BF16)
        nc.sync.dma_start(a_sb, a_s.rearrange("(k p) m -> p k m", p=128))

        # stage 3a: outer = a^T b
        with tc.tile_pool(name="p3a", bufs=1) as p3a, \
             tc.tile_pool(name="stg", bufs=2) as stgp:
            for jh in range(2):
                bts = []
                for k in range(8):
                    bt = p3a.tile([128, COPM, 192], BF16, tag=f"bt{k}")
                    nc.sync.dma_start(
                        bt, b_h[k * 128:(k + 1) * 128, :, jh * 192:(jh + 1) * 192])
                    bts.append(bt)
                for m in range(12):
                    stage = stgp.tile([128, 16, 384], BF16, tag="stg")
                    for n in range(16):
                        po = ps.tile([128, 384], F32, tag="mm")
                        bt_f = bts[0]  # placeholder for lints
                        for k in range(8):
                            nc.tensor.matmul(
                                po, a_sb[:, k, m * 128:(m + 1) * 128],
                                bts[k].rearrange("p c j -> p (c j)")[:, n * 384:(n + 1) * 384],
                                start=(k == 0), stop=(k == 7))
                        nc.any.tensor_copy(stage[:, n, :], po)
                    od = outer_d.ap().rearrange("m (d j) -> m d j", j=N)[
                        m * 128:(m + 1) * 128, :, jh * 192:(jh + 1) * 192]
                    nc.sync.dma_start(
                        od, stage.rearrange("p n (dl j) -> p (n dl) j", dl=2))

        # stage 3b: z = z + outer @ woutT + b_out
        o_v = outer_d.ap().rearrange("(c il) (d j) -> c d il j", il=NI, j=N)
        with tc.tile_pool(name="p3b", bufs=2) as p3b:
            for ig in range(8):
                rts = []
                for k in range(8):
                    rt = p3b.tile([128, 6, N], BF16, tag=f"rt{k}")
                    rt_v = rt.rearrange("(a b) i j -> a b i j", a=4)
                    for cc in range(4):
                        nc.sync.dma_start(
                            rt_v[cc], o_v[k * 4 + cc, :, ig * 6:(ig + 1) * 6, :])
                    rts.append(rt)
                zin = p3b.tile([128, 18, CZ], F32, tag="zin")
                nc.sync.dma_start(zin, zt[:, ig * 18:(ig + 1) * 18, :])
                zst = p3b.tile([128, 6, 3, CZ], F32, tag="zst")
                for il in range(6):
                    p2 = ps.tile([CZ, N], F32, tag="mm2")
                    for k in range(8):
                        nc.tensor.matmul(p2, wz_sb[:, k, :], rts[k][:, il, :],
                                         start=(k == 0), stop=(k == 7))
                    o2 = p3b.tile([CZ, N], BF16, tag="o2")
                    nc.vector.tensor_scalar_add(o2, p2, bo_sb)
                    ptT = pst.tile([128, 3, CZ], BF16, tag="pt")
                    for jc in range(3):
                        nc.tensor.transpose(ptT[:, jc, :],
                                            o2[:, jc * 128:(jc + 1) * 128], ident)
                    nc.vector.tensor_tensor(zst[:, il, :, :], ptT,
                                            zin[:, il * 3:(il + 1) * 3, :],
                                            op=ALU.add)
                nc.sync.dma_start(zot[:, ig * 18:(ig + 1) * 18, :],
                                  zst.rearrange("p a b c -> p (a b) c"))
    nc.compile()
    return nc


_cache = {}


def _get(name, builder):
    if name not in _cache:
        _cache[name] = builder()
    return _cache[name]


def _host_reference(inp):
    """numpy fallback for general msa_mask; mirrors reference.py."""
    def _ln(x, g, b):
        mu = x.mean(-1, keepdims=True)
        xc = x - mu
        var = (xc * xc).mean(-1, keepdims=True)
        return xc / np.sqrt(var + 1e-5) * g + b
    z, m = inp['z'].astype(np.float64), inp['m'].astype(np.float64)
    tm, mm = inp['token_mask'], inp['msa_mask']
    bias = _ln(z, inp['ln_z_g'], inp['ln_z_b']) @ inp['w_proj_z'].T
    mn = _ln(m, inp['ln_m_g'], inp['ln_m_b'])
    v = (mn @ inp['w_proj_m'].T).reshape(B, S, N, H, CH)
    logits = bias.transpose(0, 3, 1, 2) + (1.0 - tm[:, None]) * (-1e6)
    logits = logits - logits.max(-1, keepdims=True)
    e = np.exp(logits)
    w_att = e / e.sum(-1, keepdims=True)
    gate = 1.0 / (1.0 + np.exp(-(mn @ inp['w_proj_g'].T)))
    o = np.einsum('bhij,bsjhd->bsihd', w_att, v).reshape(B, S, N, H * CH)
    m = m + (gate * o) @ inp['w_proj_o'].T
    t = _ln(m, inp['ln_t_g'], inp['ln_t_b'])
    f1 = t @ inp['w_fc1'].T
    m = m + ((f1 / (1.0 + np.exp(-f1))) * (t @ inp['w_fc2'].T)) @ inp['w_fc3'].T
    me = mm[..., None]
    mo = _ln(m, inp['ln_o_g'], inp['ln_o_b'])
    a = (mo @ inp['w_a'].T) * me
    b2 = (mo @ inp['w_b'].T) * me
    num = np.clip(np.einsum('bsi,bsj->bij', mm, mm), 1.0, None)[..., None]
    outer = np.einsum('bsic,bsjd->bijcd', a, b2).reshape(B, N, N, COPM * COPM) / num
    z = z + outer @ inp['w_out'].T + inp['b_out']
    return z.astype(np.float32), m.astype(np.float32)


def kernel(**inputs):
    inp = {k: np.asarray(v) for k, v in inputs.items()}
    if not np.all(inp['msa_mask'] == 1.0):
        return _host_reference(inp)
    z, m = inp['z'], inp['m']
    cores = list(range(NCORE))

    # ---------- L1 ----------
    nc1 = _get('l1', build_l1)
    wz_eff = np.ascontiguousarray(
        (inp['w_proj_z'] * inp['ln_z_g'][None, :]).T).astype(BF)
    bz_eff = (inp['w_proj_z'] @ inp['ln_z_b']).astype(np.float32).reshape(H, 1)
    maskb = ((1.0 - inp['token_mask'][0]) * (-1e6)).astype(np.float32)
    maps1 = []
    for c in cores:
        i0 = c * NI
        maps1.append({
            'zr': np.ascontiguousarray(z[0, i0:i0 + NI]).reshape(-1, CZ),
            'mb': np.ascontiguousarray(maskb[i0:i0 + NI]).reshape(-1),
            'wz': wz_eff, 'bz': bz_eff,
        })
    r1 = run_bass_kernel_spmd(nc1, maps1, core_ids=cores).results
    w_att = np.concatenate([r['watt'].reshape(H, NI, N) for r in r1], axis=1)
    w_attT = w_att.transpose(0, 2, 1)  # [h, j, i]
    wt_l = np.ascontiguousarray(
        w_attT.reshape(H, 3, 128, N).transpose(2, 1, 0, 3)).astype(BF).reshape(128, -1)

    # ---------- L2 ----------
    nc2 = _get('l2', build_l2)
    g = inp['ln_m_g']
    wvg = np.ascontiguousarray(np.concatenate(
        [(inp['w_proj_m'] * g[None, :]).T,
         (inp['w_proj_g'] * g[None, :]).T], axis=1)).astype(BF)
    wo_T = np.ascontiguousarray(inp['w_proj_o'].T).astype(BF)
    gt = inp['ln_t_g']
    w12 = np.ascontiguousarray(np.concatenate(
        [(inp['w_fc1'] * gt[None, :]).T,
         (inp['w_fc2'] * gt[None, :]).T], axis=1)).astype(BF)
    w3_T = np.ascontiguousarray(inp['w_fc3'].T).astype(BF)
    go_ = inp['ln_o_g']
    wab = np.ascontiguousarray(np.concatenate(
        [(inp['w_a'] * go_[None, :]).T / float(S),
         (inp['w_b'] * go_[None, :]).T], axis=1)).astype(BF)
    maps2 = []
    for c in cores:
        s0 = c * SL
        maps2.append({
            'ms': np.ascontiguousarray(m[0, s0:s0 + SL]).reshape(-1, CM),
            'wt': wt_l, 'wvg': wvg, 'wo': wo_T, 'w12': w12, 'w3': w3_T,
            'wab': wab,
        })
    r2 = run_bass_kernel_spmd(nc2, maps2, core_ids=cores).results
    m_new = np.concatenate([r['mo_d'].reshape(SL, N, CM) for r in r2], axis=0)[None]
    a_h = np.concatenate([r['a_d'] for r in r2], axis=0)
    b_h = np.concatenate([r['b_d'] for r in r2], axis=0)

    # ---------- L3 ----------
    nc3 = _get('l3', build_l3)
    woutT = np.ascontiguousarray(inp['w_out'].T).astype(BF)
    bo = inp['b_out'].astype(np.float32).reshape(CZ, 1)
    maps3 = []
    for c in cores:
        i0 = c * NI
        maps3.append({
            'a_s': np.ascontiguousarray(a_h[:, :, i0:i0 + NI]).reshape(S, -1),
            'b_h': b_h,
            'zr': np.ascontiguousarray(z[0, i0:i0 + NI]).reshape(-1, CZ),
            'wz': woutT, 'bo': bo,
        })
    r3 = run_bass_kernel_spmd(nc3, maps3, core_ids=cores).results
    z_new = np.concatenate([r['zo'].reshape(NI, N, CZ) for r in r3], axis=0)[None]
    return z_new.astype(np.float32), m_new.astype(np.float32)


# revision 12
# speedup vs baseline: 1.0459x; 1.0459x over previous
"""DAP MSA layer (PWA + SwiGLU transition + outer-product-mean) on 8 trn2 cores.

Three SPMD launches:
  L1 (row-sharded over N):  w_att = softmax(proj_z(norm(z)) + mask)
  L2 (seq-sharded over S):  full m pipeline -> m_new, a, b
  L3 (row-sharded over N):  outer-product-mean + out-proj -> z_new
Matmuls in bf16 (fp32 PSUM accumulate), element-wise/LN/softmax in fp32.
"""
import os
import time
import numpy as np
import ml_dtypes
from contextlib import ExitStack

_VERB = os.environ.get("KERNEL_VERBOSE", "") == "1"


def _tlog(tag, t0):
    if _VERB:
        print(f"  [{tag}] {time.time()-t0:.2f}s", flush=True)
    return time.time()

import concourse.bass as bass
import concourse.tile as tile
from concourse import bacc, mybir
from concourse.bass_utils import run_bass_kernel_spmd
from concourse.masks import make_identity

F32 = mybir.dt.float32
BF16 = mybir.dt.bfloat16
AX = mybir.AxisListType.X
ALU = mybir.AluOpType
ACTF = mybir.ActivationFunctionType
BF = ml_dtypes.bfloat16

B, S, N = 1, 1024, 384
CZ, CM, H, CH, COPM, HID = 128, 64, 8, 32, 32, 256
NCORE = 8
NI = N // NCORE      # 48 z-rows per core (L1, L3)
SL = S // NCORE      # 128 msa-rows per core (L2)
SC = 16              # s-rows per L2 chunk
T = SC * 3           # 48 token-tiles per L2 chunk
HD = H * CH          # 256


def _ln_stats(nc, pool, xbuf, t_cnt, c_dim, eps):
    """xbuf [128, t_cnt, c_dim] f32 -> (negmu, rstd) each [128, t_cnt]."""
    negsum = pool.tile([128, t_cnt], F32, tag="st_a")
    nc.vector.tensor_reduce(negsum, xbuf[:, :, :], axis=AX, op=ALU.add, negate=True)
    sq = pool.tile([128, t_cnt * c_dim], BF16, tag="st_sq")
    nc.scalar.activation(sq, xbuf.rearrange("p t c -> p (t c)"), ACTF.Square)
    s2 = pool.tile([128, t_cnt], F32, tag="st_b")
    nc.vector.tensor_reduce(s2, sq.rearrange("p (t c) -> p t c", c=c_dim),
                            axis=AX, op=ALU.add)
    negmu = pool.tile([128, t_cnt], F32, tag="st_c")
    nc.vector.tensor_scalar_mul(negmu, negsum, 1.0 / c_dim)
    var = pool.tile([128, t_cnt], F32, tag="st_d")
    nc.vector.tensor_scalar_mul(var, s2, 1.0 / c_dim)
    mu2 = pool.tile([128, t_cnt], F32, tag="st_e")
    nc.vector.tensor_mul(mu2, negmu, negmu)
    nc.vector.tensor_tensor(var, var, mu2, op=ALU.subtract)
    sd = pool.tile([128, t_cnt], F32, tag="st_f")
    nc.scalar.activation(sd, var, ACTF.Sqrt, bias=eps)
    rstd = pool.tile([128, t_cnt], F32, tag="st_g")
    nc.vector.reciprocal(rstd, sd)
    return negmu, rstd


def build_l1():
    nc = bacc.Bacc("TRN2", target_bir_lowering=False, debug=False, num_devices=NCORE)
    TOK = NI * N
    zr = nc.dram_tensor("zr", [TOK, CZ], F32, kind="ExternalInput").ap()
    mb = nc.dram_tensor("mb", [TOK], F32, kind="ExternalInput").ap()
    wz = nc.dram_tensor("wz", [CZ, H], BF16, kind="ExternalInput").ap()
    bz = nc.dram_tensor("bz", [H, 1], F32, kind="ExternalInput").ap()
    watt = nc.dram_tensor("watt", [H, TOK], F32, kind="ExternalOutput").ap()

    zt = zr.rearrange("(t p) c -> p t c", p=128)
    NT = 12
    NCH = (TOK // 128) // NT  # 12

    with tile.TileContext(nc) as tc, ExitStack() as ctx:
        const = ctx.enter_context(tc.tile_pool(name="const", bufs=1))
        pool = ctx.enter_context(tc.tile_pool(name="p", bufs=2))
        big = ctx.enter_context(tc.tile_pool(name="big", bufs=1))
        ps = ctx.enter_context(tc.tile_pool(name="ps", bufs=4, space="PSUM"))
        pst = ctx.enter_context(tc.tile_pool(name="pst", bufs=2, space="PSUM"))

        ident = const.tile([128, 128], BF16)
        make_identity(nc, ident)
        wz_sb = const.tile([CZ, H], BF16)
        nc.sync.dma_start(wz_sb, wz)
        bz_sb = const.tile([H, 1], F32)
        nc.sync.dma_start(bz_sb, bz)
        eps = const.tile([128, 1], F32)
        nc.vector.memset(eps, 1e-5)

        logits = big.tile([H, TOK], F32)

        for ch in range(NCH):
            zc = pool.tile([128, NT, CZ], F32, tag="zc")
            nc.sync.dma_start(zc, zt[:, ch * NT:(ch + 1) * NT, :])
            negmu, rstd = _ln_stats(nc, pool, zc, NT, CZ, eps)
            xh = pool.tile([128, NT, CZ], BF16, tag="xh")
            for t in range(NT):
                nc.vector.tensor_scalar(xh[:, t, :], zc[:, t, :],
                                        negmu[:, t:t + 1], rstd[:, t:t + 1],
                                        op0=ALU.add, op1=ALU.mult)
            xhT = pool.tile([128, NT, 128], BF16, tag="xhT")
            for t in range(NT):
                pt = pst.tile([128, 128], BF16, tag="pt")
                nc.tensor.transpose(pt, xh[:, t, :], ident)
                nc.any.tensor_copy(xhT[:, t, :], pt)
            # mask-bias chunk, broadcast to 8 partitions
            mbc = pool.tile([H, NT * 128], F32, tag="mbc")
            mb_sl = mb[ch * NT * 128:(ch + 1) * NT * 128]
            mb_b = bass.AP(tensor=mb_sl.tensor, offset=mb_sl.offset,
                           ap=[[0, H]] + list(mb_sl.ap))
            nc.gpsimd.dma_start(out=mbc, in_=mb_b)
            xhT_f = xhT.rearrange("p t f -> p (t f)")
            for g in range(NT // 4):
                pb = ps.tile([H, 512], F32, tag="pb")
                nc.tensor.matmul(pb, wz_sb, xhT_f[:, g * 512:(g + 1) * 512],
                                 start=True, stop=True)
                off = ch * NT * 128 + g * 512
                nc.vector.tensor_scalar_add(logits[:, off:off + 512], pb, bz_sb)
                nc.vector.tensor_tensor(logits[:, off:off + 512],
                                        logits[:, off:off + 512],
                                        mbc[:, g * 512:(g + 1) * 512], op=ALU.add)

        ex = big.tile([H, TOK], F32)
        nc.scalar.activation(ex, logits, ACTF.Exp)
        ex_v = ex.rearrange("h (i j) -> h i j", j=N)
        sums = pool.tile([H, NI], F32, tag="sm")
        nc.vector.tensor_reduce(sums, ex_v, axis=AX, op=ALU.add)
        rec = pool.tile([H, NI], F32, tag="rc")
        nc.vector.reciprocal(rec, sums)
        for i in range(NI):
            nc.vector.tensor_scalar_mul(ex_v[:, i, :], ex_v[:, i, :],
                                        rec[:, i:i + 1])
        nc.sync.dma_start(watt, ex)
    nc.compile()
    return nc


def build_l2():
    nc = bacc.Bacc("TRN2", target_bir_lowering=False, debug=False, num_devices=NCORE)
    TOKL = SL * N
    ms = nc.dram_tensor("ms", [TOKL, CM], F32, kind="ExternalInput").ap()
    wt = nc.dram_tensor("wt", [128, 3 * H * N], BF16, kind="ExternalInput").ap()
    wvg = nc.dram_tensor("wvg", [CM, 2 * HD], BF16, kind="ExternalInput").ap()
    wo = nc.dram_tensor("wo", [HD, CM], BF16, kind="ExternalInput").ap()
    w12 = nc.dram_tensor("w12", [CM, 2 * HID], BF16, kind="ExternalInput").ap()
    w3 = nc.dram_tensor("w3", [HID, CM], BF16, kind="ExternalInput").ap()
    wab = nc.dram_tensor("wab", [CM, 2 * COPM], BF16, kind="ExternalInput").ap()
    mo_d = nc.dram_tensor("mo_d", [TOKL, CM], F32, kind="ExternalOutput").ap()
    a_d = nc.dram_tensor("a_d", [SL, COPM, N], BF16, kind="ExternalOutput").ap()
    b_d = nc.dram_tensor("b_d", [SL, COPM, N], BF16, kind="ExternalOutput").ap()

    mt = ms.rearrange("(t p) c -> p t c", p=128)
    mot = mo_d.rearrange("(t p) c -> p t c", p=128)

    with tile.TileContext(nc) as tc, ExitStack() as ctx:
        const = ctx.enter_context(tc.tile_pool(name="const", bufs=1))
        pool = ctx.enter_context(tc.tile_pool(name="p", bufs=2))
        buf1 = ctx.enter_context(tc.tile_pool(name="b1", bufs=1))
        ps = ctx.enter_context(tc.tile_pool(name="ps", bufs=2, space="PSUM"))
        pst = ctx.enter_context(tc.tile_pool(name="pst", bufs=2, space="PSUM"))

        ident = const.tile([128, 128], BF16)
        make_identity(nc, ident)
        eps = const.tile([128, 1], F32)
        nc.vector.memset(eps, 1e-5)
        wvg_sb = const.tile([CM, 2 * HD], BF16)
        nc.sync.dma_start(wvg_sb, wvg)
        wo_sb = const.tile([128, 2, CM], BF16)
        nc.sync.dma_start(wo_sb, wo.rearrange("(k p) c -> p k c", p=128))
        w12_sb = const.tile([CM, 2 * HID], BF16)
        nc.sync.dma_start(w12_sb, w12)
        w3_sb = const.tile([128, 2, CM], BF16)
        nc.sync.dma_start(w3_sb, w3.rearrange("(k p) c -> p k c", p=128))
        wab_sb = const.tile([CM, 2 * COPM], BF16)
        nc.sync.dma_start(wab_sb, wab)
        wt_sb = const.tile([128, 3, H, N], BF16)
        nc.sync.dma_start(wt_sb, wt.rearrange("p (c h i) -> p c h i", c=3, h=H))

        def ln_block(xbuf, out_bf, out_T):
            negmu, rstd = _ln_stats(nc, pool, xbuf, T, CM, eps)
            for t in range(T):
                nc.vector.tensor_scalar(out_bf[:, t, :], xbuf[:, t, :],
                                        negmu[:, t:t + 1], rstd[:, t:t + 1],
                                        op0=ALU.add, op1=ALU.mult)
            for t in range(T):
                pt = pst.tile([CM, 128], BF16, tag="pt")
                nc.tensor.transpose(pt, out_bf[:, t, :], ident)
                nc.any.tensor_copy(out_T[:, t, :], pt)

        for ch in range(SL // SC):
            mbuf = pool.tile([128, T, CM], F32, tag="mbuf")
            nc.sync.dma_start(mbuf, mt[:, ch * T:(ch + 1) * T, :])
            xh = buf1.tile([128, T, CM], BF16, tag="xh")
            mnT = buf1.tile([CM, T, 128], BF16, tag="mnT")
            ln_block(mbuf, xh, mnT)

            # ---- v | gate ----
            vbuf = buf1.tile([128, 3, SC, HD], BF16, tag="vbuf")
            gbuf = buf1.tile([128, 3, SC, HD], BF16, tag="gbuf")
            for sl in range(SC):
                for ic in range(3):
                    pv = ps.tile([128, 512], F32, tag="mm")
                    nc.tensor.matmul(pv, mnT[:, sl * 3 + ic, :], wvg_sb,
                                     start=True, stop=True)
                    nc.vector.tensor_copy(vbuf[:, ic, sl, :], pv[:, :HD])
                    nc.scalar.activation(gbuf[:, ic, sl, :], pv[:, HD:],
                                         ACTF.Sigmoid)

            # ---- o = att @ v ; go = gate*o ; transpose go ----
            goT = buf1.tile([128, 2, SC, 3, 128], BF16, tag="goT")
            for ic in range(3):
                go = buf1.tile([128, SC, HD], BF16, tag="go")
                for hh in range(H):
                    po = ps.tile([128, 512], F32, tag="mm")
                    for jc in range(3):
                        nc.tensor.matmul(
                            po, wt_sb[:, jc, hh, ic * 128:(ic + 1) * 128],
                            vbuf[:, jc, :, hh * CH:(hh + 1) * CH],
                            start=(jc == 0), stop=(jc == 2))
                    nc.vector.tensor_tensor(
                        go[:, :, hh * CH:(hh + 1) * CH],
                        po.rearrange("p (s d) -> p s d", d=CH),
                        gbuf[:, ic, :, hh * CH:(hh + 1) * CH], op=ALU.mult)
                for sl in range(SC):
                    ptT = pst.tile([128, 2, 128], BF16, tag="pt")
                    nc.tensor.transpose(ptT[:, 0, :], go[:, sl, 0:128], ident)
                    nc.tensor.transpose(ptT[:, 1, :], go[:, sl, 128:256], ident)
                    nc.any.tensor_copy(goT[:, 0, sl, ic, :], ptT[:, 0, :])
                    nc.any.tensor_copy(goT[:, 1, sl, ic, :], ptT[:, 1, :])

            # ---- m1 = m + go @ woT ----
            m1 = buf1.tile([128, T, CM], F32, tag="m1")
            goT_f = goT.rearrange("p h s c f -> p h (s c f)")
            for n in range(12):
                pd = ps.tile([CM, 512], F32, tag="mmd")
                for k in range(2):
                    nc.tensor.matmul(pd, wo_sb[:, k, :],
                                     goT_f[:, k, n * 512:(n + 1) * 512],
                                     start=(k == 0), stop=(k == 1))
                dsb = pool.tile([CM, 512], BF16, tag="dsb")
                nc.any.tensor_copy(dsb, pd)
                ptb = pst.tile([128, 4, CM], BF16, tag="pt")
                for q in range(4):
                    nc.tensor.transpose(ptb[:, q, :], dsb[:, q * 128:(q + 1) * 128],
                                        ident[:CM, :CM])
                nc.vector.tensor_tensor(m1[:, n * 4:(n + 1) * 4, :], ptb,
                                        mbuf[:, n * 4:(n + 1) * 4, :], op=ALU.add)

            # ---- transition ----
            xh2 = buf1.tile([128, T, CM], BF16, tag="xh")
            tT = buf1.tile([CM, T, 128], BF16, tag="mnT")
            ln_block(m1, xh2, tT)
            tT_f = tT.rearrange("c t f -> c (t f)")
            hm = buf1.tile([128, 2, T * 128], BF16, tag="vbuf")
            for g in range(12):
                sw = pool.tile([128, 2, 512], BF16, tag="sw")
                for hc in range(2):
                    pf = ps.tile([128, 512], F32, tag="mm")
                    nc.tensor.matmul(pf, w12_sb[:, hc * 128:(hc + 1) * 128],
                                     tT_f[:, g * 512:(g + 1) * 512],
                                     start=True, stop=True)
                    nc.scalar.activation(sw[:, hc, :], pf, ACTF.Silu)
                for hc in range(2):
                    pf = ps.tile([128, 512], F32, tag="mm")
                    nc.tensor.matmul(pf, w12_sb[:, 256 + hc * 128:256 + (hc + 1) * 128],
                                     tT_f[:, g * 512:(g + 1) * 512],
                                     start=True, stop=True)
                    nc.vector.tensor_tensor(hm[:, hc, g * 512:(g + 1) * 512],
                                            sw[:, hc, :], pf, op=ALU.mult)
            m2 = pool.tile([128, T, CM], F32, tag="mbuf")
            for n in range(12):
                pd = ps.tile([CM, 512], F32, tag="mmd")
                for k in range(2):
                    nc.tensor.matmul(pd, w3_sb[:, k, :],
                                     hm[:, k, n * 512:(n + 1) * 512],
                                     start=(k == 0), stop=(k == 1))
                dsb = pool.tile([CM, 512], BF16, tag="dsb")
                nc.any.tensor_copy(dsb, pd)
                ptb = pst.tile([128, 4, CM], BF16, tag="pt")
                for q in range(4):
                    nc.tensor.transpose(ptb[:, q, :], dsb[:, q * 128:(q + 1) * 128],
                                        ident[:CM, :CM])
                nc.vector.tensor_tensor(m2[:, n * 4:(n + 1) * 4, :], ptb,
                                        m1[:, n * 4:(n + 1) * 4, :], op=ALU.add)
            nc.sync.dma_start(mot[:, ch * T:(ch + 1) * T, :], m2)

            # ---- a|b ----
            xh3 = buf1.tile([128, T, CM], BF16, tag="xh")
            moT = buf1.tile([CM, T, 128], BF16, tag="mnT")
            ln_block(m2, xh3, moT)
            moT_f = moT.rearrange("c t f -> c (t f)")
            ab = buf1.tile([2 * COPM, T * 128], BF16, tag="gbuf")
            for g in range(12):
                pa = ps.tile([2 * COPM, 512], F32, tag="mmd")
                nc.tensor.matmul(pa, wab_sb, moT_f[:, g * 512:(g + 1) * 512],
                                 start=True, stop=True)
                nc.any.tensor_copy(ab[:, g * 512:(g + 1) * 512], pa)
            ab_v = ab.rearrange("c (s i) -> c s i", i=N)
            nc.sync.dma_start(
                a_d[ch * SC:(ch + 1) * SC, :, :].rearrange("s c i -> c s i"),
                ab_v[0:COPM, :, :])
            nc.sync.dma_start(
                b_d[ch * SC:(ch + 1) * SC, :, :].rearrange("s c i -> c s i"),
                ab_v[COPM:, :, :])
    nc.compile()
    return nc


def build_l3():
    nc = bacc.Bacc("TRN2", target_bir_lowering=False, debug=False, num_devices=NCORE)
    TOK = NI * N
    a_s = nc.dram_tensor("a_s", [S, COPM * NI], BF16, kind="ExternalInput").ap()
    b_h = nc.dram_tensor("b_h", [S, COPM, N], BF16, kind="ExternalInput").ap()
    zr = nc.dram_tensor("zr", [TOK, CZ], F32, kind="ExternalInput").ap()
    wz = nc.dram_tensor("wz", [COPM * COPM, CZ], BF16, kind="ExternalInput").ap()
    bo = nc.dram_tensor("bo", [CZ, 1], F32, kind="ExternalInput").ap()
    outer_d = nc.dram_tensor("outer_d", [COPM * NI, COPM * N], BF16)
    zo = nc.dram_tensor("zo", [TOK, CZ], F32, kind="ExternalOutput").ap()

    zt = zr.rearrange("(t p) c -> p t c", p=128)
    zot = zo.rearrange("(t p) c -> p t c", p=128)

    with tile.TileContext(nc) as tc, ExitStack() as ctx:
        const = ctx.enter_context(tc.tile_pool(name="const", bufs=1))
        ps = ctx.enter_context(tc.tile_pool(name="ps", bufs=3, space="PSUM"))
        pst = ctx.enter_context(tc.tile_pool(name="pst", bufs=2, space="PSUM"))

        ident = const.tile([128, 128], BF16)
        make_identity(nc, ident)
        wz_sb = const.tile([128, 8, CZ], BF16)
        nc.sync.dma_start(wz_sb, wz.rearrange("(k p) z -> p k z", p=128))
        bo_sb = const.tile([CZ, 1], F32)
        nc.sync.dma_start(bo_sb, bo)
        a_sb = const.tile([128, 8, COPM * NI], BF16)
        nc.sync.dma_start(a_sb, a_s.rearrange("(k p) m -> p k m", p=128))

        # stage 3a: outer = a^T b
        with tc.tile_pool(name="p3a", bufs=1) as p3a, \
             tc.tile_pool(name="stg", bufs=2) as stgp:
            for jh in range(2):
                bts = []
                for k in range(8):
                    bt = p3a.tile([128, COPM, 192], BF16, tag=f"bt{k}")
                    nc.sync.dma_start(
                        bt, b_h[k * 128:(k + 1) * 128, :, jh * 192:(jh + 1) * 192])
                    bts.append(bt)
                for m in range(12):
                    stage = stgp.tile([128, 16, 384], BF16, tag="stg")
                    for n in range(16):
                        po = ps.tile([128, 384], F32, tag="mm")
                        bt_f = bts[0]  # placeholder for lints
                        for k in range(8):
                            nc.tensor.matmul(
                                po, a_sb[:, k, m * 128:(m + 1) * 128],
                                bts[k].rearrange("p c j -> p (c j)")[:, n * 384:(n + 1) * 384],
                                start=(k == 0), stop=(k == 7))
                        nc.any.tensor_copy(stage[:, n, :], po)
                    od = outer_d.ap().rearrange("m (d j) -> m d j", j=N)[
                        m * 128:(m + 1) * 128, :, jh * 192:(jh + 1) * 192]
                    nc.sync.dma_start(
                        od, stage.rearrange("p n (dl j) -> p (n dl) j", dl=2))

        # stage 3b: z = z + outer @ woutT + b_out
        o_v = outer_d.ap().rearrange("(c il) (d j) -> c d il j", il=NI, j=N)
        with tc.tile_pool(name="p3b", bufs=2) as p3b:
            for ig in range(8):
                rts = []
                for k in range(8):
                    rt = p3b.tile([128, 6, N], BF16, tag=f"rt{k}")
                    rt_v = rt.rearrange("(a b) i j -> a b i j", a=4)
                    for cc in range(4):
                        nc.sync.dma_start(
                            rt_v[cc], o_v[k * 4 + cc, :, ig * 6:(ig + 1) * 6, :])
                    rts.append(rt)
                zin = p3b.tile([128, 18, CZ], F32, tag="zin")
                nc.sync.dma_start(zin, zt[:, ig * 18:(ig + 1) * 18, :])
                zst = p3b.tile([128, 6, 3, CZ], F32, tag="zst")
                for il in range(6):
                    p2 = ps.tile([CZ, N], F32, tag="mm2")
                    for k in range(8):
                        nc.tensor.matmul(p2, wz_sb[:, k, :], rts[k][:, il, :],
                                         start=(k == 0), stop=(k == 7))
                    o2 = p3b.tile([CZ, N], BF16, tag="o2")
                    nc.vector.tensor_scalar_add(o2, p2, bo_sb)
                    ptT = pst.tile([128, 3, CZ], BF16, tag="pt")
                    for jc in range(3):
                        nc.tensor.transpose(ptT[:, jc, :],
                                            o2[:, jc * 128:(jc + 1) * 128], ident)
                    nc.vector.tensor_tensor(zst[:, il, :, :], ptT,
                                            zin[:, il * 3:(il + 1) * 3, :],
                                            op=ALU.add)
                nc.sync.dma_start(zot[:, ig * 18:(ig + 1) * 18, :],
                                  zst.rearrange("p a b c -> p (a b) c"))
    nc.compile()
    return nc


_cache = {}


def _get(name, builder):
    if name not in _cache:
        _cache[name] = builder()
    return _cache[name]


def _host_reference(inp):
    """numpy fallback for general msa_mask; mirrors reference.py."""
    def _ln(x, g, b):
        mu = x.mean(-1, keepdims=True)
        xc = x - mu
        var = (xc * xc).mean(-1, keepdims=True)
        return xc / np.sqrt(var + 1e-5) * g + b
    z, m = inp['z'].astype(np.float64), inp['m'].astype(np.float64)
    tm, mm = inp['token_mask'], inp['msa_mask']
    bias = _ln(z, inp['ln_z_g'], inp['ln_z_b']) @ inp['w_proj_z'].T
    mn = _ln(m, inp['ln_m_g'], inp['ln_m_b'])
    v = (mn @ inp['w_proj_m'].T).reshape(B, S, N, H, CH)
    logits = bias.transpose(0, 3, 1, 2) + (1.0 - tm[:, None]) * (-1e6)
    logits = logits - logits.max(-1, keepdims=True)
    e = np.exp(logits)
    w_att = e / e.sum(-1, keepdims=True)
    gate = 1.0 / (1.0 + np.exp(-(mn @ inp['w_proj_g'].T)))
    o = np.einsum('bhij,bsjhd->bsihd', w_att, v).reshape(B, S, N, H * CH)
    m = m + (gate * o) @ inp['w_proj_o'].T
    t = _ln(m, inp['ln_t_g'], inp['ln_t_b'])
    f1 = t @ inp['w_fc1'].T
    m = m + ((f1 / (1.0 + np.exp(-f1))) * (t @ inp['w_fc2'].T)) @ inp['w_fc3'].T
    me = mm[..., None]
    mo = _ln(m, inp['ln_o_g'], inp['ln_o_b'])
    a = (mo @ inp['w_a'].T) * me
    b2 = (mo @ inp['w_b'].T) * me
    num = np.clip(np.einsum('bsi,bsj->bij', mm, mm), 1.0, None)[..., None]
    outer = np.einsum('bsic,bsjd->bijcd', a, b2).reshape(B, N, N, COPM * COPM) / num
    z = z + outer @ inp['w_out'].T + inp['b_out']
    return z.astype(np.float32), m.astype(np.float32)


def kernel(**inputs):
    inp = {k: np.asarray(v) for k, v in inputs.items()}
    if not np.all(inp['msa_mask'] == 1.0):
        return _host_reference(inp)
    z, m = inp['z'], inp['m']
    cores = list(range(NCORE))

    # ---------- L1 ----------
    nc1 = _get('l1', build_l1)
    wz_eff = np.ascontiguousarray(
        (inp['w_proj_z'] * inp['ln_z_g'][None, :]).T).astype(BF)
    bz_eff = (inp['w_proj_z'] @ inp['ln_z_b']).astype(np.float32).reshape(H, 1)
    maskb = ((1.0 - inp['token_mask'][0]) * (-1e6)).astype(np.float32)
    maps1 = []
    for c in cores:
        i0 = c * NI
        maps1.append({
            'zr': np.ascontiguousarray(z[0, i0:i0 + NI]).reshape(-1, CZ),
            'mb': np.ascontiguousarray(maskb[i0:i0 + NI]).reshape(-1),
            'wz': wz_eff, 'bz': bz_eff,
        })
    t0 = time.time()
    r1 = run_bass_kernel_spmd(nc1, maps1, core_ids=cores).results
    t0 = _tlog("L1", t0)
    w_att = np.concatenate([r['watt'].reshape(H, NI, N) for r in r1], axis=1)
    w_attT = w_att.transpose(0, 2, 1)  # [h, j, i]
    wt_l = np.ascontiguousarray(
        w_attT.reshape(H, 3, 128, N).transpose(2, 1, 0, 3)).astype(BF).reshape(128, -1)

    # ---------- L2 ----------
    nc2 = _get('l2', build_l2)
    g = inp['ln_m_g']
    wvg = np.ascontiguousarray(np.concatenate(
        [(inp['w_proj_m'] * g[None, :]).T,
         (inp['w_proj_g'] * g[None, :]).T], axis=1)).astype(BF)
    wo_T = np.ascontiguousarray(inp['w_proj_o'].T).astype(BF)
    gt = inp['ln_t_g']
    w12 = np.ascontiguousarray(np.concatenate(
        [(inp['w_fc1'] * gt[None, :]).T,
         (inp['w_fc2'] * gt[None, :]).T], axis=1)).astype(BF)
    w3_T = np.ascontiguousarray(inp['w_fc3'].T).astype(BF)
    go_ = inp['ln_o_g']
    wab = np.ascontiguousarray(np.concatenate(
        [(inp['w_a'] * go_[None, :]).T / float(S),
         (inp['w_b'] * go_[None, :]).T], axis=1)).astype(BF)
    maps2 = []
    for c in cores:
        s0 = c * SL
        maps2.append({
            'ms': np.ascontiguousarray(m[0, s0:s0 + SL]).reshape(-1, CM),
            'wt': wt_l, 'wvg': wvg, 'wo': wo_T, 'w12': w12, 'w3': w3_T,
            'wab': wab,
        })
    t0 = time.time()
    r2 = run_bass_kernel_spmd(nc2, maps2, core_ids=cores).results
    t0 = _tlog("L2", t0)
    m_new = np.concatenate([r['mo_d'].reshape(SL, N, CM) for r in r2], axis=0)[None]
    a_h = np.concatenate([r['a_d'] for r in r2], axis=0)
    b_h = np.concatenate([r['b_d'] for r in r2], axis=0)

    # ---------- L3 ----------
    nc3 = _get('l3', build_l3)
    woutT = np.ascontiguousarray(inp['w_out'].T).astype(BF)
    bo = inp['b_out'].astype(np.float32).reshape(CZ, 1)
    maps3 = []
    for c in cores:
        i0 = c * NI
        maps3.append({
            'a_s': np.ascontiguousarray(a_h[:, :, i0:i0 + NI]).reshape(S, -1),
            'b_h': b_h,
            'zr': np.ascontiguousarray(z[0, i0:i0 + NI]).reshape(-1, CZ),
            'wz': woutT, 'bo': bo,
        })
    t0 = time.time()
    r3 = run_bass_kernel_spmd(nc3, maps3, core_ids=cores).results
    t0 = _tlog("L3", t0)
    z_new = np.concatenate([r['zo'].reshape(NI, N, CZ) for r in r3], axis=0)[None]
    return z_new.astype(np.float32), m_new.astype(np.float32)


# revision 13
# speedup vs baseline: 1.5202x; 1.4535x over previous
"""DAP MSA layer (PWA + SwiGLU transition + outer-product-mean) on 8 trn2 cores.

Three SPMD launches:
  L1 (row-sharded over N):  w_att = softmax(proj_z(norm(z)) + mask)
  L2 (seq-sharded over S):  full m pipeline -> m_new, a, b
  L3 (row-sharded over N):  outer-product-mean + out-proj -> z_new
Matmuls in bf16 (fp32 PSUM accumulate), element-wise/LN/softmax in fp32.
"""
import os
import time
import numpy as np
import ml_dtypes
from contextlib import ExitStack

_VERB = os.environ.get("KERNEL_VERBOSE", "") == "1"


def _tlog(tag, t0):
    if _VERB:
        print(f"  [{tag}] {time.time()-t0:.2f}s", flush=True)
    return time.time()

import concourse.bass as bass
import concourse.tile as tile
from concourse import bacc, mybir
from concourse.bass_utils import run_bass_kernel_spmd
from concourse.masks import make_identity

F32 = mybir.dt.float32
BF16 = mybir.dt.bfloat16
AX = mybir.AxisListType.X
ALU = mybir.AluOpType
ACTF = mybir.ActivationFunctionType
BF = ml_dtypes.bfloat16

B, S, N = 1, 1024, 384
CZ, CM, H, CH, COPM, HID = 128, 64, 8, 32, 32, 256
NCORE = 8
NI = N // NCORE      # 48 z-rows per core (L1, L3)
SL = S // NCORE      # 128 msa-rows per core (L2)
SC = 16              # s-rows per L2 chunk
T = SC * 3           # 48 token-tiles per L2 chunk
HD = H * CH          # 256


def _ln_stats(nc, pool, xbuf, t_cnt, c_dim, eps):
    """xbuf [128, t_cnt, c_dim] f32 -> (negmu, rstd) each [128, t_cnt]."""
    negsum = pool.tile([128, t_cnt], F32, tag="st_a")
    nc.vector.tensor_reduce(negsum, xbuf[:, :, :], axis=AX, op=ALU.add, negate=True)
    sq = pool.tile([128, t_cnt * c_dim], BF16, tag="st_sq")
    nc.scalar.activation(sq, xbuf.rearrange("p t c -> p (t c)"), ACTF.Square)
    s2 = pool.tile([128, t_cnt], F32, tag="st_b")
    nc.vector.tensor_reduce(s2, sq.rearrange("p (t c) -> p t c", c=c_dim),
                            axis=AX, op=ALU.add)
    negmu = pool.tile([128, t_cnt], F32, tag="st_c")
    nc.vector.tensor_scalar_mul(negmu, negsum, 1.0 / c_dim)
    var = pool.tile([128, t_cnt], F32, tag="st_d")
    nc.vector.tensor_scalar_mul(var, s2, 1.0 / c_dim)
    mu2 = pool.tile([128, t_cnt], F32, tag="st_e")
    nc.vector.tensor_mul(mu2, negmu, negmu)
    nc.vector.tensor_tensor(var, var, mu2, op=ALU.subtract)
    sd = pool.tile([128, t_cnt], F32, tag="st_f")
    nc.scalar.activation(sd, var, ACTF.Sqrt, bias=eps)
    rstd = pool.tile([128, t_cnt], F32, tag="st_g")
    nc.vector.reciprocal(rstd, sd)
    return negmu, rstd


def build_l1():
    nc = bacc.Bacc("TRN2", target_bir_lowering=False, debug=False, num_devices=NCORE)
    TOK = NI * N
    zr = nc.dram_tensor("zr", [TOK, CZ], F32, kind="ExternalInput").ap()
    mb = nc.dram_tensor("mb", [TOK], F32, kind="ExternalInput").ap()
    wz = nc.dram_tensor("wz", [CZ, H], BF16, kind="ExternalInput").ap()
    bz = nc.dram_tensor("bz", [H, 1], F32, kind="ExternalInput").ap()
    watt = nc.dram_tensor("watt", [H, TOK], F32, kind="ExternalOutput").ap()

    zt = zr.rearrange("(t p) c -> p t c", p=128)
    NT = 12
    NCH = (TOK // 128) // NT  # 12

    with tile.TileContext(nc) as tc, ExitStack() as ctx:
        const = ctx.enter_context(tc.tile_pool(name="const", bufs=1))
        pool = ctx.enter_context(tc.tile_pool(name="p", bufs=2))
        big = ctx.enter_context(tc.tile_pool(name="big", bufs=1))
        ps = ctx.enter_context(tc.tile_pool(name="ps", bufs=4, space="PSUM"))
        pst = ctx.enter_context(tc.tile_pool(name="pst", bufs=2, space="PSUM"))

        ident = const.tile([128, 128], BF16)
        make_identity(nc, ident)
        wz_sb = const.tile([CZ, H], BF16)
        nc.sync.dma_start(wz_sb, wz)
        bz_sb = const.tile([H, 1], F32)
        nc.sync.dma_start(bz_sb, bz)
        eps = const.tile([128, 1], F32)
        nc.vector.memset(eps, 1e-5)

        logits = big.tile([H, TOK], F32)

        for ch in range(NCH):
            zc = pool.tile([128, NT, CZ], F32, tag="zc")
            nc.sync.dma_start(zc, zt[:, ch * NT:(ch + 1) * NT, :])
            negmu, rstd = _ln_stats(nc, pool, zc, NT, CZ, eps)
            xh = pool.tile([128, NT, CZ], BF16, tag="xh")
            for t in range(NT):
                nc.vector.tensor_scalar(xh[:, t, :], zc[:, t, :],
                                        negmu[:, t:t + 1], rstd[:, t:t + 1],
                                        op0=ALU.add, op1=ALU.mult)
            xhT = pool.tile([128, NT, 128], BF16, tag="xhT")
            for t in range(NT):
                pt = pst.tile([128, 128], BF16, tag="pt")
                nc.tensor.transpose(pt, xh[:, t, :], ident)
                nc.any.tensor_copy(xhT[:, t, :], pt)
            # mask-bias chunk, broadcast to 8 partitions
            mbc = pool.tile([H, NT * 128], F32, tag="mbc")
            mb_sl = mb[ch * NT * 128:(ch + 1) * NT * 128]
            mb_b = bass.AP(tensor=mb_sl.tensor, offset=mb_sl.offset,
                           ap=[[0, H]] + list(mb_sl.ap))
            nc.gpsimd.dma_start(out=mbc, in_=mb_b)
            xhT_f = xhT.rearrange("p t f -> p (t f)")
            for g in range(NT // 4):
                pb = ps.tile([H, 512], F32, tag="pb")
                nc.tensor.matmul(pb, wz_sb, xhT_f[:, g * 512:(g + 1) * 512],
                                 start=True, stop=True)
                off = ch * NT * 128 + g * 512
                nc.vector.tensor_scalar_add(logits[:, off:off + 512], pb, bz_sb)
                nc.vector.tensor_tensor(logits[:, off:off + 512],
                                        logits[:, off:off + 512],
                                        mbc[:, g * 512:(g + 1) * 512], op=ALU.add)

        ex = big.tile([H, TOK], F32)
        nc.scalar.activation(ex, logits, ACTF.Exp)
        ex_v = ex.rearrange("h (i j) -> h i j", j=N)
        sums = pool.tile([H, NI], F32, tag="sm")
        nc.vector.tensor_reduce(sums, ex_v, axis=AX, op=ALU.add)
        rec = pool.tile([H, NI], F32, tag="rc")
        nc.vector.reciprocal(rec, sums)
        for i in range(NI):
            nc.vector.tensor_scalar_mul(ex_v[:, i, :], ex_v[:, i, :],
                                        rec[:, i:i + 1])
        nc.sync.dma_start(watt, ex)
    nc.compile()
    return nc


def build_l2():
    nc = bacc.Bacc("TRN2", target_bir_lowering=False, debug=False, num_devices=NCORE)
    TOKL = SL * N
    ms = nc.dram_tensor("ms", [TOKL, CM], F32, kind="ExternalInput").ap()
    wt = nc.dram_tensor("wt", [128, 3 * H * N], BF16, kind="ExternalInput").ap()
    wvg = nc.dram_tensor("wvg", [CM, 2 * HD], BF16, kind="ExternalInput").ap()
    wo = nc.dram_tensor("wo", [HD, CM], BF16, kind="ExternalInput").ap()
    w12 = nc.dram_tensor("w12", [CM, 2 * HID], BF16, kind="ExternalInput").ap()
    w3 = nc.dram_tensor("w3", [HID, CM], BF16, kind="ExternalInput").ap()
    wab = nc.dram_tensor("wab", [CM, 2 * COPM], BF16, kind="ExternalInput").ap()
    mo_d = nc.dram_tensor("mo_d", [TOKL, CM], BF16, kind="ExternalOutput").ap()
    a_d = nc.dram_tensor("a_d", [SL, COPM, N], BF16, kind="ExternalOutput").ap()
    b_d = nc.dram_tensor("b_d", [SL, COPM, N], BF16, kind="ExternalOutput").ap()

    mt = ms.rearrange("(t p) c -> p t c", p=128)
    mot = mo_d.rearrange("(t p) c -> p t c", p=128)

    with tile.TileContext(nc) as tc, ExitStack() as ctx:
        const = ctx.enter_context(tc.tile_pool(name="const", bufs=1))
        pool = ctx.enter_context(tc.tile_pool(name="p", bufs=2))
        buf1 = ctx.enter_context(tc.tile_pool(name="b1", bufs=1))
        ps = ctx.enter_context(tc.tile_pool(name="ps", bufs=2, space="PSUM"))
        pst = ctx.enter_context(tc.tile_pool(name="pst", bufs=2, space="PSUM"))

        ident = const.tile([128, 128], BF16)
        make_identity(nc, ident)
        eps = const.tile([128, 1], F32)
        nc.vector.memset(eps, 1e-5)
        wvg_sb = const.tile([CM, 2 * HD], BF16)
        nc.sync.dma_start(wvg_sb, wvg)
        wo_sb = const.tile([128, 2, CM], BF16)
        nc.sync.dma_start(wo_sb, wo.rearrange("(k p) c -> p k c", p=128))
        w12_sb = const.tile([CM, 2 * HID], BF16)
        nc.sync.dma_start(w12_sb, w12)
        w3_sb = const.tile([128, 2, CM], BF16)
        nc.sync.dma_start(w3_sb, w3.rearrange("(k p) c -> p k c", p=128))
        wab_sb = const.tile([CM, 2 * COPM], BF16)
        nc.sync.dma_start(wab_sb, wab)
        wt_sb = const.tile([128, 3, H, N], BF16)
        nc.sync.dma_start(wt_sb, wt.rearrange("p (c h i) -> p c h i", c=3, h=H))

        def ln_block(xbuf, out_bf, out_T):
            negmu, rstd = _ln_stats(nc, pool, xbuf, T, CM, eps)
            for t in range(T):
                nc.vector.tensor_scalar(out_bf[:, t, :], xbuf[:, t, :],
                                        negmu[:, t:t + 1], rstd[:, t:t + 1],
                                        op0=ALU.add, op1=ALU.mult)
            for t in range(T):
                pt = pst.tile([CM, 128], BF16, tag="pt")
                nc.tensor.transpose(pt, out_bf[:, t, :], ident)
                nc.any.tensor_copy(out_T[:, t, :], pt)

        for ch in range(SL // SC):
            mbuf = pool.tile([128, T, CM], F32, tag="mbuf")
            nc.sync.dma_start(mbuf, mt[:, ch * T:(ch + 1) * T, :])
            xh = buf1.tile([128, T, CM], BF16, tag="xh")
            mnT = buf1.tile([CM, T, 128], BF16, tag="mnT")
            ln_block(mbuf, xh, mnT)

            # ---- v | gate ----
            vbuf = buf1.tile([128, 3, SC, HD], BF16, tag="vbuf")
            gbuf = buf1.tile([128, 3, SC, HD], BF16, tag="gbuf")
            for sl in range(SC):
                for ic in range(3):
                    pv = ps.tile([128, 512], F32, tag="mm")
                    nc.tensor.matmul(pv, mnT[:, sl * 3 + ic, :], wvg_sb,
                                     start=True, stop=True)
                    nc.vector.tensor_copy(vbuf[:, ic, sl, :], pv[:, :HD])
                    nc.scalar.activation(gbuf[:, ic, sl, :], pv[:, HD:],
                                         ACTF.Sigmoid)

            # ---- o = att @ v ; go = gate*o ; transpose go ----
            goT = buf1.tile([128, 2, SC, 3, 128], BF16, tag="goT")
            for ic in range(3):
                go = buf1.tile([128, SC, HD], BF16, tag="go")
                for hh in range(H):
                    po = ps.tile([128, 512], F32, tag="mm")
                    for jc in range(3):
                        nc.tensor.matmul(
                            po, wt_sb[:, jc, hh, ic * 128:(ic + 1) * 128],
                            vbuf[:, jc, :, hh * CH:(hh + 1) * CH],
                            start=(jc == 0), stop=(jc == 2))
                    nc.vector.tensor_tensor(
                        go[:, :, hh * CH:(hh + 1) * CH],
                        po.rearrange("p (s d) -> p s d", d=CH),
                        gbuf[:, ic, :, hh * CH:(hh + 1) * CH], op=ALU.mult)
                for sl in range(SC):
                    ptT = pst.tile([128, 2, 128], BF16, tag="pt")
                    nc.tensor.transpose(ptT[:, 0, :], go[:, sl, 0:128], ident)
                    nc.tensor.transpose(ptT[:, 1, :], go[:, sl, 128:256], ident)
                    nc.any.tensor_copy(goT[:, 0, sl, ic, :], ptT[:, 0, :])
                    nc.any.tensor_copy(goT[:, 1, sl, ic, :], ptT[:, 1, :])

            # ---- m1 = m + go @ woT ----
            m1 = buf1.tile([128, T, CM], F32, tag="m1")
            goT_f = goT.rearrange("p h s c f -> p h (s c f)")
            for n in range(12):
                pd = ps.tile([CM, 512], F32, tag="mmd")
                for k in range(2):
                    nc.tensor.matmul(pd, wo_sb[:, k, :],
                                     goT_f[:, k, n * 512:(n + 1) * 512],
                                     start=(k == 0), stop=(k == 1))
                dsb = pool.tile([CM, 512], BF16, tag="dsb")
                nc.any.tensor_copy(dsb, pd)
                ptb = pst.tile([128, 4, CM], BF16, tag="pt")
                for q in range(4):
                    nc.tensor.transpose(ptb[:, q, :], dsb[:, q * 128:(q + 1) * 128],
                                        ident[:CM, :CM])
                nc.vector.tensor_tensor(m1[:, n * 4:(n + 1) * 4, :], ptb,
                                        mbuf[:, n * 4:(n + 1) * 4, :], op=ALU.add)

            # ---- transition ----
            xh2 = buf1.tile([128, T, CM], BF16, tag="xh")
            tT = buf1.tile([CM, T, 128], BF16, tag="mnT")
            ln_block(m1, xh2, tT)
            tT_f = tT.rearrange("c t f -> c (t f)")
            hm = buf1.tile([128, 2, T * 128], BF16, tag="vbuf")
            for g in range(12):
                sw = pool.tile([128, 2, 512], BF16, tag="sw")
                for hc in range(2):
                    pf = ps.tile([128, 512], F32, tag="mm")
                    nc.tensor.matmul(pf, w12_sb[:, hc * 128:(hc + 1) * 128],
                                     tT_f[:, g * 512:(g + 1) * 512],
                                     start=True, stop=True)
                    nc.scalar.activation(sw[:, hc, :], pf, ACTF.Silu)
                for hc in range(2):
                    pf = ps.tile([128, 512], F32, tag="mm")
                    nc.tensor.matmul(pf, w12_sb[:, 256 + hc * 128:256 + (hc + 1) * 128],
                                     tT_f[:, g * 512:(g + 1) * 512],
                                     start=True, stop=True)
                    nc.vector.tensor_tensor(hm[:, hc, g * 512:(g + 1) * 512],
                                            sw[:, hc, :], pf, op=ALU.mult)
            m2 = pool.tile([128, T, CM], F32, tag="mbuf")
            for n in range(12):
                pd = ps.tile([CM, 512], F32, tag="mmd")
                for k in range(2):
                    nc.tensor.matmul(pd, w3_sb[:, k, :],
                                     hm[:, k, n * 512:(n + 1) * 512],
                                     start=(k == 0), stop=(k == 1))
                dsb = pool.tile([CM, 512], BF16, tag="dsb")
                nc.any.tensor_copy(dsb, pd)
                ptb = pst.tile([128, 4, CM], BF16, tag="pt")
                for q in range(4):
                    nc.tensor.transpose(ptb[:, q, :], dsb[:, q * 128:(q + 1) * 128],
                                        ident[:CM, :CM])
                nc.vector.tensor_tensor(m2[:, n * 4:(n + 1) * 4, :], ptb,
                                        m1[:, n * 4:(n + 1) * 4, :], op=ALU.add)
            dm = pool.tile([128, T, CM], BF16, tag="dmout")
            nc.vector.tensor_tensor(dm, m2, mbuf, op=ALU.subtract)
            nc.sync.dma_start(mot[:, ch * T:(ch + 1) * T, :], dm)

            # ---- a|b ----
            xh3 = buf1.tile([128, T, CM], BF16, tag="xh")
            moT = buf1.tile([CM, T, 128], BF16, tag="mnT")
            ln_block(m2, xh3, moT)
            moT_f = moT.rearrange("c t f -> c (t f)")
            ab = buf1.tile([2 * COPM, T * 128], BF16, tag="gbuf")
            for g in range(12):
                pa = ps.tile([2 * COPM, 512], F32, tag="mmd")
                nc.tensor.matmul(pa, wab_sb, moT_f[:, g * 512:(g + 1) * 512],
                                 start=True, stop=True)
                nc.any.tensor_copy(ab[:, g * 512:(g + 1) * 512], pa)
            ab_v = ab.rearrange("c (s i) -> c s i", i=N)
            nc.sync.dma_start(
                a_d[ch * SC:(ch + 1) * SC, :, :].rearrange("s c i -> c s i"),
                ab_v[0:COPM, :, :])
            nc.sync.dma_start(
                b_d[ch * SC:(ch + 1) * SC, :, :].rearrange("s c i -> c s i"),
                ab_v[COPM:, :, :])
    nc.compile()
    return nc


def build_l3():
    nc = bacc.Bacc("TRN2", target_bir_lowering=False, debug=False, num_devices=NCORE)
    TOK = NI * N
    a_s = nc.dram_tensor("a_s", [S, COPM * NI], BF16, kind="ExternalInput").ap()
    b_h = nc.dram_tensor("b_h", [S, COPM, N], BF16, kind="ExternalInput").ap()
    wz = nc.dram_tensor("wz", [COPM * COPM, CZ], BF16, kind="ExternalInput").ap()
    bo = nc.dram_tensor("bo", [CZ, 1], F32, kind="ExternalInput").ap()
    outer_d = nc.dram_tensor("outer_d", [COPM * NI, COPM * N], BF16)
    zo = nc.dram_tensor("zo", [TOK, CZ], BF16, kind="ExternalOutput").ap()

    zot = zo.rearrange("(t p) c -> p t c", p=128)

    with tile.TileContext(nc) as tc, ExitStack() as ctx:
        const = ctx.enter_context(tc.tile_pool(name="const", bufs=1))
        ps = ctx.enter_context(tc.tile_pool(name="ps", bufs=3, space="PSUM"))
        pst = ctx.enter_context(tc.tile_pool(name="pst", bufs=2, space="PSUM"))

        ident = const.tile([128, 128], BF16)
        make_identity(nc, ident)
        wz_sb = const.tile([128, 8, CZ], BF16)
        nc.sync.dma_start(wz_sb, wz.rearrange("(k p) z -> p k z", p=128))
        bo_sb = const.tile([CZ, 1], F32)
        nc.sync.dma_start(bo_sb, bo)
        a_sb = const.tile([128, 8, COPM * NI], BF16)
        nc.sync.dma_start(a_sb, a_s.rearrange("(k p) m -> p k m", p=128))

        # stage 3a: outer = a^T b
        with tc.tile_pool(name="p3a", bufs=1) as p3a, \
             tc.tile_pool(name="stg", bufs=2) as stgp:
            for jh in range(2):
                bts = []
                for k in range(8):
                    bt = p3a.tile([128, COPM, 192], BF16, tag=f"bt{k}")
                    nc.sync.dma_start(
                        bt, b_h[k * 128:(k + 1) * 128, :, jh * 192:(jh + 1) * 192])
                    bts.append(bt)
                for m in range(12):
                    stage = stgp.tile([128, 16, 384], BF16, tag="stg")
                    for n in range(16):
                        po = ps.tile([128, 384], F32, tag="mm")
                        bt_f = bts[0]  # placeholder for lints
                        for k in range(8):
                            nc.tensor.matmul(
                                po, a_sb[:, k, m * 128:(m + 1) * 128],
                                bts[k].rearrange("p c j -> p (c j)")[:, n * 384:(n + 1) * 384],
                                start=(k == 0), stop=(k == 7))
                        nc.any.tensor_copy(stage[:, n, :], po)
                    od = outer_d.ap().rearrange("m (d j) -> m d j", j=N)[
                        m * 128:(m + 1) * 128, :, jh * 192:(jh + 1) * 192]
                    nc.sync.dma_start(
                        od, stage.rearrange("p n (dl j) -> p (n dl) j", dl=2))

        # stage 3b: z = z + outer @ woutT + b_out
        o_v = outer_d.ap().rearrange("(c il) (d j) -> c d il j", il=NI, j=N)
        with tc.tile_pool(name="p3b", bufs=2) as p3b:
            for ig in range(8):
                rts = []
                for k in range(8):
                    rt = p3b.tile([128, 6, N], BF16, tag=f"rt{k}")
                    rt_v = rt.rearrange("(a b) i j -> a b i j", a=4)
                    for cc in range(4):
                        nc.sync.dma_start(
                            rt_v[cc], o_v[k * 4 + cc, :, ig * 6:(ig + 1) * 6, :])
                    rts.append(rt)
                zst = p3b.tile([128, 6, 3, CZ], BF16, tag="zst")
                for il in range(6):
                    p2 = ps.tile([CZ, N], F32, tag="mm2")
                    for k in range(8):
                        nc.tensor.matmul(p2, wz_sb[:, k, :], rts[k][:, il, :],
                                         start=(k == 0), stop=(k == 7))
                    o2 = p3b.tile([CZ, N], BF16, tag="o2")
                    nc.vector.tensor_scalar_add(o2, p2, bo_sb)
                    ptT = pst.tile([128, 3, CZ], BF16, tag="pt")
                    for jc in range(3):
                        nc.tensor.transpose(ptT[:, jc, :],
                                            o2[:, jc * 128:(jc + 1) * 128], ident)
                    nc.any.tensor_copy(zst[:, il, :, :], ptT)
                nc.sync.dma_start(zot[:, ig * 18:(ig + 1) * 18, :],
                                  zst.rearrange("p a b c -> p (a b) c"))
    nc.compile()
    return nc


_cache = {}


def _get(name, builder):
    if name not in _cache:
        _cache[name] = builder()
    return _cache[name]


def _host_reference(inp):
    """numpy fallback for general msa_mask; mirrors reference.py."""
    def _ln(x, g, b):
        mu = x.mean(-1, keepdims=True)
        xc = x - mu
        var = (xc * xc).mean(-1, keepdims=True)
        return xc / np.sqrt(var + 1e-5) * g + b
    z, m = inp['z'].astype(np.float64), inp['m'].astype(np.float64)
    tm, mm = inp['token_mask'], inp['msa_mask']
    bias = _ln(z, inp['ln_z_g'], inp['ln_z_b']) @ inp['w_proj_z'].T
    mn = _ln(m, inp['ln_m_g'], inp['ln_m_b'])
    v = (mn @ inp['w_proj_m'].T).reshape(B, S, N, H, CH)
    logits = bias.transpose(0, 3, 1, 2) + (1.0 - tm[:, None]) * (-1e6)
    logits = logits - logits.max(-1, keepdims=True)
    e = np.exp(logits)
    w_att = e / e.sum(-1, keepdims=True)
    gate = 1.0 / (1.0 + np.exp(-(mn @ inp['w_proj_g'].T)))
    o = np.einsum('bhij,bsjhd->bsihd', w_att, v).reshape(B, S, N, H * CH)
    m = m + (gate * o) @ inp['w_proj_o'].T
    t = _ln(m, inp['ln_t_g'], inp['ln_t_b'])
    f1 = t @ inp['w_fc1'].T
    m = m + ((f1 / (1.0 + np.exp(-f1))) * (t @ inp['w_fc2'].T)) @ inp['w_fc3'].T
    me = mm[..., None]
    mo = _ln(m, inp['ln_o_g'], inp['ln_o_b'])
    a = (mo @ inp['w_a'].T) * me
    b2 = (mo @ inp['w_b'].T) * me
    num = np.clip(np.einsum('bsi,bsj->bij', mm, mm), 1.0, None)[..., None]
    outer = np.einsum('bsic,bsjd->bijcd', a, b2).reshape(B, N, N, COPM * COPM) / num
    z = z + outer @ inp['w_out'].T + inp['b_out']
    return z.astype(np.float32), m.astype(np.float32)


def kernel(**inputs):
    inp = {k: np.asarray(v) for k, v in inputs.items()}
    if not np.all(inp['msa_mask'] == 1.0):
        return _host_reference(inp)
    z, m = inp['z'], inp['m']
    cores = list(range(NCORE))

    # ---------- L1 ----------
    nc1 = _get('l1', build_l1)
    wz_eff = np.ascontiguousarray(
        (inp['w_proj_z'] * inp['ln_z_g'][None, :]).T).astype(BF)
    bz_eff = (inp['w_proj_z'] @ inp['ln_z_b']).astype(np.float32).reshape(H, 1)
    maskb = ((1.0 - inp['token_mask'][0]) * (-1e6)).astype(np.float32)
    maps1 = []
    for c in cores:
        i0 = c * NI
        maps1.append({
            'zr': np.ascontiguousarray(z[0, i0:i0 + NI]).reshape(-1, CZ),
            'mb': np.ascontiguousarray(maskb[i0:i0 + NI]).reshape(-1),
            'wz': wz_eff, 'bz': bz_eff,
        })
    t0 = time.time()
    r1 = run_bass_kernel_spmd(nc1, maps1, core_ids=cores).results
    t0 = _tlog("L1", t0)
    w_att = np.concatenate([r['watt'].reshape(H, NI, N) for r in r1], axis=1)
    w_attT = w_att.transpose(0, 2, 1)  # [h, j, i]
    wt_l = np.ascontiguousarray(
        w_attT.reshape(H, 3, 128, N).transpose(2, 1, 0, 3)).astype(BF).reshape(128, -1)

    # ---------- L2 ----------
    nc2 = _get('l2', build_l2)
    g = inp['ln_m_g']
    wvg = np.ascontiguousarray(np.concatenate(
        [(inp['w_proj_m'] * g[None, :]).T,
         (inp['w_proj_g'] * g[None, :]).T], axis=1)).astype(BF)
    wo_T = np.ascontiguousarray(inp['w_proj_o'].T).astype(BF)
    gt = inp['ln_t_g']
    w12 = np.ascontiguousarray(np.concatenate(
        [(inp['w_fc1'] * gt[None, :]).T,
         (inp['w_fc2'] * gt[None, :]).T], axis=1)).astype(BF)
    w3_T = np.ascontiguousarray(inp['w_fc3'].T).astype(BF)
    go_ = inp['ln_o_g']
    wab = np.ascontiguousarray(np.concatenate(
        [(inp['w_a'] * go_[None, :]).T / float(S),
         (inp['w_b'] * go_[None, :]).T], axis=1)).astype(BF)
    maps2 = []
    for c in cores:
        s0 = c * SL
        maps2.append({
            'ms': np.ascontiguousarray(m[0, s0:s0 + SL]).reshape(-1, CM),
            'wt': wt_l, 'wvg': wvg, 'wo': wo_T, 'w12': w12, 'w3': w3_T,
            'wab': wab,
        })
    t0 = time.time()
    r2 = run_bass_kernel_spmd(nc2, maps2, core_ids=cores).results
    t0 = _tlog("L2", t0)
    m_delta = np.concatenate([r['mo_d'].reshape(SL, N, CM) for r in r2],
                             axis=0)[None].astype(np.float32)
    m_new = m.astype(np.float32) + m_delta
    a_h = np.concatenate([r['a_d'] for r in r2], axis=0)
    b_h = np.concatenate([r['b_d'] for r in r2], axis=0)

    # ---------- L3 ----------
    nc3 = _get('l3', build_l3)
    woutT = np.ascontiguousarray(inp['w_out'].T).astype(BF)
    bo = inp['b_out'].astype(np.float32).reshape(CZ, 1)
    maps3 = []
    for c in cores:
        i0 = c * NI
        maps3.append({
            'a_s': np.ascontiguousarray(a_h[:, :, i0:i0 + NI]).reshape(S, -1),
            'b_h': b_h,
            'wz': woutT, 'bo': bo,
        })
    t0 = time.time()
    r3 = run_bass_kernel_spmd(nc3, maps3, core_ids=cores).results
    t0 = _tlog("L3", t0)
    z_delta = np.concatenate([r['zo'].reshape(NI, N, CZ) for r in r3],
                             axis=0)[None].astype(np.float32)
    z_new = z.astype(np.float32) + z_delta
    return z_new, m_new
